# revision 3
# baseline (speedup 1.0000x reference)
"""Trainium2 Bass kernel for ConditionalEdgeDenoiser (GNN edge MLP denoiser).

Reference computation (per batch b, nodes i,j):
    h = concat([edge_t[b,i,j,:],            # 4   (EC)
                node_ctx[b,i,:],            # 80  (src = x_cond||code_cond)
                node_ctx[b,j,:],            # 80  (dst)
                time_emb[b,:]])             # 128 (TDIM)
    h1 = silu(h @ W1 + b1); h2 = silu(h1 @ W2 + b2); out = (h2 @ W3 + b3) * mask

Strategy (8 cores, data-parallel over (B x row-halves) = 8 shards of 128 rows):
  * Layer 1 is one augmented matmul per hid-half: stationary stacks
    [W1_edge (4); W1_dst (80); srcbias rows (RPT)], moving stacks
    [edge_T (4); node_ctx_T (80); row-indicators (RPT)], where
    srcbias = node_ctx[i] @ W1_src + time_emb @ W1_time + b1 is host-side
    (0.1% of FLOPs).
  * Activation engine is the bottleneck (silu at 1 elem/cycle/partition), so
    ops are as large as PSUM allows: silu1 is ONE [128, 2048] op over a
    4-bank psum; silu2 is 2 half ops with the b2 bias fused.
  * Layer 3 runs transposed: stationary = h2 slice (128 edges), moving = W3
    [128, 4], so PSUM gets [128 edges, 4] per chunk.  Both the PE cost
    (64 instead of 2048 rows/tile) and the DVE bias+mask cost (free size 32
    instead of 3072) collapse.  The tiny L3 psum aliases into the Y slot.
  * Emission is a 2.5-stage software pipeline: L1(k)+silu1(k),
    L2a(k-1)+silu2a(k-1), L3+mask+out(k-2), L2b(k-1)+silu2b(k-1) so the Act
    engine runs back-to-back while psum single-buffering stays race-free.
"""

import os
import sys

sys.path.insert(0, "/opt/trn_rl_repo")
os.environ.setdefault("MYCRO_LOCAL_CACHE", "1")

import numpy as np

import concourse.bass as bass  # noqa: E402
import concourse.mybir as mybir  # noqa: E402
import concourse.tile as tile  # noqa: E402
from concourse import bacc  # noqa: E402
from concourse.bass_utils import run_bass_kernel_spmd  # noqa: E402

B, N, EC, FEAT, CODE, HID, TDIM = 4, 256, 4, 64, 16, 256, 128
NCTX = FEAT + CODE  # 80
NCORES = 8
RPT = 4                      # grid rows per tile
E = RPT * N                  # 1024 edge columns per tile
CH = 512                     # matmul moving-dim chunk (fp32 PSUM bank limit)
NCH = E // CH                # chunks per tile
ROWS = N // 2                # 128 grid rows per core
NT = ROWS // RPT             # 32 tiles per core
KAUG = EC + NCTX + RPT       # 88 = augmented contraction dim for layer 1
OW = 8 * EC                  # 32 = out cols per tile in [128, 8, EC] layout

F32 = mybir.dt.float32
F32R = mybir.dt.float32r
AF = mybir.ActivationFunctionType

_CACHE = {}


def _build():
    nc = bacc.Bacc("TRN2", debug=False, num_devices=NCORES)

    # ---- DRAM I/O (per core) ----
    edge_d = nc.dram_tensor("edge", [NT, EC, E], F32R, kind="ExternalInput")
    srcb_d = nc.dram_tensor("srcb", [ROWS, HID], F32R, kind="ExternalInput")
    w1ed_d = nc.dram_tensor("w1ed", [EC + NCTX, HID], F32R, kind="ExternalInput")
    b2c_d = nc.dram_tensor("b2c", [128, 2], F32, kind="ExternalInput")
    b3rep_d = nc.dram_tensor("b3rep", [128, OW], F32, kind="ExternalInput")
    maskf_d = nc.dram_tensor("maskf", [128, NT * OW], F32, kind="ExternalInput")
    w2_d = nc.dram_tensor("w2", [HID, HID], F32R, kind="ExternalInput")
    w3_d = nc.dram_tensor("w3", [HID, EC], F32R, kind="ExternalInput")
    rhsstat_d = nc.dram_tensor("rhsstat", [NCTX + RPT, E], F32R, kind="ExternalInput")
    out_d = nc.dram_tensor("out", [NT, 128, OW], F32, kind="ExternalOutput")

    with tile.TileContext(nc) as tc:
        with tc.tile_pool(name="const", bufs=1) as cp, \
             tc.tile_pool(name="h", bufs=2) as hp, \
             tc.tile_pool(name="o", bufs=3) as op, \
             tc.tile_pool(name="ps", bufs=1, space="PSUM") as pp:

            # ---------- augmented layer-1 operands (ping-pong pairs) ----------
            # Buffer 0 loads first so tile 0's critical path (edge/srcb DMAs
            # emitted at k==0) is right behind them in the SP/HWDGE queues;
            # buffer 1's static loads are emitted at the end of k==0.
            lh = [None, None]
            rhs_t = [None, None]
            for q in range(2):
                lh[q] = cp.tile([KAUG, HID], F32R, tag=f"lh{q}", name=f"lh{q}")
                rhs_t[q] = cp.tile([KAUG, E], F32R, tag=f"rhs{q}", name=f"rhs{q}")
            nc.sync.dma_start(out=lh[0][0:EC + NCTX, :], in_=w1ed_d[:])
            nc.sync.dma_start(out=rhs_t[0][EC:KAUG, :], in_=rhsstat_d[:])

            # ---------- PE p-state warm-up ----------
            # The cost of a matmul is set by the clock ramp at DISPATCH time.
            # Anchor the ramp with immediate tiny matmuls, then park further
            # tiny matmuls in the 4-deep wait queue gated on the w1ed /
            # rhsstat loads: tile 0's real matmuls then dispatch ~4us after
            # the anchor (ramp fully warm) instead of at t~0 (cold clock).
            pdum = pp.tile([32, 384], F32, name="pdum", tag="p1")
            for r in range(2):      # anchor: gated on the w1ed DMA (~2.5us)
                nc.tensor.matmul(pdum[:, 64 * r:64 * (r + 1)],
                                 lhsT=lh[0][0:32, 0:32],
                                 rhs=lh[0][0:32, 0:64], start=True, stop=True)
            for r in range(2, 6):   # parked on the rhsstat DMA (~4us)
                nc.tensor.matmul(pdum[:, 64 * r:64 * (r + 1)],
                                 lhsT=rhs_t[0][32:64, 0:32],
                                 rhs=rhs_t[0][32:64, 0:64], start=True, stop=True)

            # ---------- main loop: software pipeline ----------
            w2k0 = w2k1 = w30 = w31 = b2c = b3rep = maskf = None  # noqa
            h1s, h2s = {}, {}
            for k in range(NT + 2):
                if k == 1:
                    # layer-2/3 constants: SWDGE (Pool) keeps them off the
                    # HWDGE queue that feeds tile 0/1's edge loads
                    w2k0 = cp.tile([128, HID], F32R, tag="w2k0", name="w2k0")
                    nc.gpsimd.dma_start(out=w2k0, in_=w2_d[0:128])
                    w2k1 = cp.tile([128, HID], F32R, tag="w2k1", name="w2k1")
                    nc.gpsimd.dma_start(out=w2k1, in_=w2_d[128:256])
                    b2c = cp.tile([128, 2], F32, tag="b2c", name="b2c")
                    nc.gpsimd.dma_start(out=b2c, in_=b2c_d[:])

                if k < NT:
                    rhs = rhs_t[k % 2]
                    nc.sync.dma_start(out=rhs[0:EC, :], in_=edge_d[k])
                    lht = lh[k % 2]
                    nc.gpsimd.dma_start(out=lht[EC + NCTX:KAUG, :],
                                        in_=srcb_d[RPT * k:RPT * (k + 1)])
                    # L1: one [128, 2E] psum, halves at cols 0:E / E:2E
                    p1 = pp.tile([128, 2 * E], F32, name=f"p1_{k}", tag="p1")
                    for h in range(2):
                        for c in range(NCH):
                            nc.tensor.matmul(
                                p1[:, h * E + c * CH:h * E + (c + 1) * CH],
                                lhsT=lht[:, h * 128:(h + 1) * 128],
                                rhs=rhs[:, c * CH:(c + 1) * CH],
                                start=True, stop=True)
                    h1 = hp.tile([128, 2 * E], F32R, tag="h1", name=f"h1_{k}")
                    nc.scalar.activation(h1, p1, AF.Silu)
                    h1s[k] = h1
                    if k == 0:
                        nc.sync.dma_start(out=lh[1][0:EC + NCTX, :], in_=w1ed_d[:])
                        nc.sync.dma_start(out=rhs_t[1][EC:KAUG, :], in_=rhsstat_d[:])

                if 1 <= k <= NT:
                    j = k - 1
                    h1 = h1s.pop(j)
                    h2 = hp.tile([128, 2 * E], F32R, tag="h2", name=f"h2_{j}")

                    # --- half a: psum X ---
                    px = pp.tile([128, E], F32, name=f"p2a_{j}", tag="p2a")
                    for c in range(NCH):
                        dst = px[:, c * CH:(c + 1) * CH]
                        nc.tensor.matmul(dst, lhsT=w2k0[:, 0:128],
                                         rhs=h1[:, c * CH:(c + 1) * CH],
                                         start=True, stop=False)
                        nc.tensor.matmul(dst, lhsT=w2k1[:, 0:128],
                                         rhs=h1[:, E + c * CH:E + (c + 1) * CH],
                                         start=False, stop=True)
                    nc.scalar.activation(h2[:, 0:E], px, AF.Silu,
                                         bias=b2c[:, 0:1])

                # --- L3 + mask + out for tile i = k-2 (between halves so
                # its psum can alias the Y slot race-free) ---
                if k >= 2:
                    i = k - 2
                    h2p = h2s.pop(i)
                    p3 = pp.tile([128, OW], F32, name=f"p3_{i}",
                                 tag="p2a" if i >= NT - 2 else "p2b")
                    for c in range(8):
                        dst = p3[:, EC * c:EC * (c + 1)]
                        nc.tensor.matmul(dst,
                                         lhsT=h2p[:, 128 * c:128 * (c + 1)],
                                         rhs=w30, start=True, stop=False)
                        nc.tensor.matmul(dst,
                                         lhsT=h2p[:, E + 128 * c:E + 128 * (c + 1)],
                                         rhs=w31, start=False, stop=True)
                    ot = op.tile([128, OW], F32, name=f"ot{i}", tag="ot")
                    nc.vector.tensor_add(out=ot, in0=p3, in1=b3rep)
                    nc.vector.tensor_mul(out=ot, in0=ot,
                                         in1=maskf[:, OW * i:OW * (i + 1)])
                    nc.sync.dma_start(out=out_d[i], in_=ot)

                if 1 <= k <= NT:
                    j = k - 1
                    # --- half b: psum Y (same tag as p3 -> serialized reuse) ---
                    py = pp.tile([128, E], F32, name=f"p2b_{j}", tag="p2b")
                    for c in range(NCH):
                        dst = py[:, c * CH:(c + 1) * CH]
                        nc.tensor.matmul(dst, lhsT=w2k0[:, 128:256],
                                         rhs=h1[:, c * CH:(c + 1) * CH],
                                         start=True, stop=False)
                        nc.tensor.matmul(dst, lhsT=w2k1[:, 128:256],
                                         rhs=h1[:, E + c * CH:E + (c + 1) * CH],
                                         start=False, stop=True)
                    nc.scalar.activation(h2[:, E:2 * E], py, AF.Silu,
                                         bias=b2c[:, 1:2])
                    h2s[j] = h2
                    if k == 1:
                        w30 = cp.tile([128, EC], F32R, tag="w30", name="w30")
                        nc.sync.dma_start(out=w30, in_=w3_d[0:128])
                        w31 = cp.tile([128, EC], F32R, tag="w31", name="w31")
                        nc.sync.dma_start(out=w31, in_=w3_d[128:256])
                        b3rep = cp.tile([128, OW], F32, tag="b3rep", name="b3rep")
                        nc.sync.dma_start(out=b3rep, in_=b3rep_d[:])
                        maskf = cp.tile([128, NT * OW], F32, tag="maskf",
                                        name="maskf")
                        nc.sync.dma_start(out=maskf, in_=maskf_d[:])

    nc.compile()
    return nc


def _get_nc():
    if "nc" not in _CACHE:
        _CACHE["nc"] = _build()
    return _CACHE["nc"]


def _time_embedding(t):
    half = TDIM // 2
    freqs = np.exp(-np.arange(half, dtype=np.float32)
                   * (np.float32(np.log(10000.0)) / np.float32(half - 1)))
    args = np.asarray(t).astype(np.float32)[:, None] * freqs[None, :]
    return np.concatenate([np.sin(args), np.cos(args)], axis=1).astype(np.float32)


def _indicator():
    ind = np.zeros((RPT, E), dtype=np.float32)
    for r in range(RPT):
        ind[r, r * N:(r + 1) * N] = 1.0
    return ind


def _prepare_in_maps(edge_t, x_cond, code_cond, t, node_mask, W1, b1, W2, b2, W3, b3):
    edge_t = np.ascontiguousarray(np.asarray(edge_t, dtype=np.float32))
    node_ctx = np.concatenate(
        [np.asarray(x_cond, np.float32), np.asarray(code_cond, np.float32)], axis=-1)
    temb = _time_embedding(t)                       # [B, TDIM]
    maskf = np.asarray(node_mask).astype(np.float32)  # [B, N]
    W1 = np.asarray(W1, np.float32)
    w1e = np.ascontiguousarray(W1[0:EC])
    w1s = W1[EC:EC + NCTX]
    w1d = np.ascontiguousarray(W1[EC + NCTX:EC + 2 * NCTX])
    w1t = W1[EC + 2 * NCTX:]
    b1 = np.asarray(b1, np.float32)
    b2c = np.ascontiguousarray(np.asarray(b2, np.float32).reshape(2, 128).T)
    b3 = np.asarray(b3, np.float32)
    W2 = np.ascontiguousarray(np.asarray(W2, np.float32))
    W3 = np.ascontiguousarray(np.asarray(W3, np.float32))
    b3rep = np.ascontiguousarray(np.tile(b3, (128, 8)))      # [128, OW]
    # srcbias (host bias precompute - 0.1% of model FLOPs): [B*N, HID]
    srcb_full = (node_ctx.reshape(B * N, NCTX) @ w1s
                 + (temb @ w1t + b1)[:, None, :].repeat(N, axis=1).reshape(B * N, HID)
                 ).astype(np.float32)

    in_maps = []
    for c in range(NCORES):
        b, ih = c // 2, c % 2
        i0 = ih * ROWS
        es = edge_t[b, i0:i0 + ROWS]               # [ROWS, N, EC]
        er = np.ascontiguousarray(
            es.reshape(NT, RPT, N, EC).transpose(0, 3, 1, 2).reshape(NT, EC, E))
        # maskfull[p, k*OW + (r*2+jh)*EC + e] = mask_i[i0+4k+r]*mask_j[jh*128+p]
        mi = maskf[b, i0:i0 + ROWS].reshape(NT, RPT)         # [k, r]
        mj = maskf[b].reshape(2, 128)                        # [jh, p]
        mfull = (mi[:, :, None, None, None]
                 * mj[None, None, :, :, None]                # [k, r, jh, p, e]
                 * np.ones((1, 1, 1, 1, EC), np.float32))
        mfull = np.ascontiguousarray(
            mfull.transpose(3, 0, 1, 2, 4).reshape(128, NT * OW))
        in_maps.append({
            "edge": er,
            "srcb": np.ascontiguousarray(srcb_full[b * N + i0:b * N + i0 + ROWS]),
            "w1ed": np.ascontiguousarray(np.vstack([w1e, w1d])),
            "b2c": b2c, "b3rep": b3rep, "maskf": mfull,
            "w2": W2, "w3": W3,
            "rhsstat": np.ascontiguousarray(
                np.vstack([np.tile(node_ctx[b].T, (1, RPT)), _indicator()])),
        })
    return in_maps


def _assemble(results):
    out = np.empty((B, N, N, EC), dtype=np.float32)
    for c in range(NCORES):
        b, ih = c // 2, c % 2
        i0 = ih * ROWS
        o = results[c]["out"]                      # [NT, 128, OW]
        o = o.reshape(NT, 128, RPT, 2, EC).transpose(0, 2, 3, 1, 4)
        out[b, i0:i0 + ROWS] = o.reshape(ROWS, N, EC)
    return out


def _run(in_maps, trace=False, **kwargs):
    nc = _get_nc()
    return run_bass_kernel_spmd(nc, in_maps, list(range(NCORES)), trace=trace, **kwargs)


def kernel(**inputs):
    in_maps = _prepare_in_maps(**inputs)
    res = _run(in_maps)
    return _assemble(res.results)
